# revision 35
# baseline (speedup 1.0000x reference)
"""Luong-style attention (B=16, T=S=E=D=1024) on 8 TRN2 NeuronCores.

Data-parallel over batch: 2 batches per core, no collectives. Per batch:

    M1   = H @ A            (T,E)     [A = W_attn]
    G    = M1 @ Enc^T       (T,S)     energies minus the row-constant H@b term
    ener = G + (H@b)[:,None]          (output attn_energies)
    W    = softmax_rows(G)            (== softmax(ener); bias is row-constant)
    C^T  = Enc(stationary) @ W^T      weighted context, transposed via DMA XBAR
    h    = tanh([C|H] @ W_out^T)      via lhsT = [C^T; H^T]

mm1/mm2 run in float32r (softmax input needs ~2^-13 operand precision),
mm3/mm4 in bf16 (fp8 was measured at 2-4.5e-2 final error — too coarse).

Structure (vs the 387us baseline):
  - mm4 is software-pipelined one (b,th) iteration late, so its 27us of PE
    work covers the softmax/W^T-transpose latency of the current iteration
    (otherwise mm3 stalls on softmax(tl3) and HAM downclocks the PE).
  - W^T via PE identity transposes, emitted one tl late so each waits on an
    already-computed softmax tile. (The DMA XBAR transpose was tried on both
    HWDGE queues: on sync it races with input loads on HW and corrupts
    wt_sb; on scalar its serialization guard stalls the ACT stream 7-20us
    per transpose. Do not revisit.)
  - Softmax works on an SBUF fp32 copy of the energies (softmax is
    shift-invariant, so ener = G + hb serves directly); the PSUM G banks
    free after one ACT copy -> psG bufs=2 suffices.
  - mm1 is dt-outer over et-pairs with per-dt-slice DMAs of A^T/H^T
    interleaved across both HWDGE queues; the PE starts after ~1.5MB lands.
  - Outputs are bf16 on the wire (host upcasts).
"""

import os
import numpy as np
import ml_dtypes

B, T, S, E, D = 16, 1024, 1024, 1024, 1024
P = 128
NCORES = 8
BPC = B // NCORES
TH = 2
THS = T // TH
ET = E // P
DT = D // P
ST = S // P
TT = T // P
CT = (E + D) // P
NTL = THS // P

BF16 = ml_dtypes.bfloat16

TRACE = bool(os.environ.get("BASS_KERNEL_TRACE"))
LAST_EXEC_NS = None
_cached = None


def _install_trace_shim():
    import sys, types
    import antenv
    if getattr(antenv, "axon_hooks", None) is not None:
        return
    mod = types.ModuleType("antenv.axon_hooks")
    state = {"hook": None}
    mod.set_axon_ntff_profile_hook = lambda h: state.__setitem__("hook", h)
    mod.get_axon_ntff_profile_hook = lambda: state["hook"]
    sys.modules["antenv.axon_hooks"] = mod
    antenv.axon_hooks = mod
    try:
        from trn_agent_boot.trn_boot import _ntff_profile_via_ctypes
        mod.set_axon_ntff_profile_hook(
            _ntff_profile_via_ctypes("/opt/axon/libaxon_pjrt.so"))
    except Exception:
        pass
    import concourse.bass_utils as bu
    bu.upload_artifacts = lambda tmpdir: "local://" + tmpdir


def _build():
    import concourse.bass as bass
    import concourse.bacc as bacc
    import concourse.mybir as mybir
    import concourse.tile as tile
    from contextlib import ExitStack

    dt = mybir.dt
    ts = bass.ts
    AF = mybir.ActivationFunctionType

    nc = bacc.Bacc("TRN2", target_bir_lowering=False, debug=False)

    A_r = nc.declare_dram_parameter("A_r", [D, E], dt.float32r, isOutput=False)
    WoT = nc.declare_dram_parameter("WoT", [E + D, D], dt.bfloat16, isOutput=False)
    HT_r = nc.declare_dram_parameter("HT_r", [BPC, D, T], dt.float32r, isOutput=False)
    HT_bf = nc.declare_dram_parameter("HT_bf", [BPC, D, T], dt.bfloat16, isOutput=False)
    EncT_r = nc.declare_dram_parameter("EncT_r", [BPC, E, S], dt.float32r, isOutput=False)
    Enc = nc.declare_dram_parameter("Enc", [BPC, S, E], dt.bfloat16, isOutput=False)
    hb = nc.declare_dram_parameter("hb", [BPC, T], dt.float32, isOutput=False)
    out_h = nc.declare_dram_parameter("out_h", [BPC, T, D], dt.bfloat16, isOutput=True)
    out_w = nc.declare_dram_parameter("out_w", [BPC, T, S], dt.bfloat16, isOutput=True)
    out_e = nc.declare_dram_parameter("out_e", [BPC, T, S], dt.bfloat16, isOutput=True)

    with tile.TileContext(nc) as tc, ExitStack() as ctx:
        from concourse.masks import make_identity

        const = ctx.enter_context(tc.tile_pool(name="const", bufs=1))
        wpool = ctx.enter_context(tc.tile_pool(name="wpool", bufs=1))
        bpool = ctx.enter_context(tc.tile_pool(name="bpool", bufs=1))
        hpool = ctx.enter_context(tc.tile_pool(name="hpool", bufs=1))
        h2pool = ctx.enter_context(tc.tile_pool(name="h2pool", bufs=2))
        work = ctx.enter_context(tc.tile_pool(name="work", bufs=2))
        work3 = ctx.enter_context(tc.tile_pool(name="work3", bufs=3))
        work4 = ctx.enter_context(tc.tile_pool(name="work4", bufs=4))
        psA = ctx.enter_context(tc.tile_pool(name="psA", bufs=2, space="PSUM"))
        psG = ctx.enter_context(tc.tile_pool(name="psG", bufs=2, space="PSUM"))
        psT = ctx.enter_context(tc.tile_pool(name="psT", bufs=2, space="PSUM"))

        ident = const.tile([P, P], dt.bfloat16)
        make_identity(nc, ident[:])
        warm_a = const.tile([P, P], dt.bfloat16)
        warm_r = const.tile([P, 512], dt.bfloat16)
        nc.gpsimd.memset(warm_a[:], 0.0)
        nc.gpsimd.memset(warm_r[:], 0.0)

        def warm(n):
            # HAM/idle filler: keeps the PE streaming while DMAs land.
            wp = psA.tile([P, 512], dt.float32, tag="psA")
            for wi in range(n):
                nc.tensor.matmul(wp[:], warm_a[:], warm_r[:],
                                 start=(wi == 0), stop=(wi == n - 1))

        warm(24)

        # --- startup-critical DMAs, interleaved across the two HWDGE queues
        a_r = wpool.tile([P, DT, E], dt.float32r)
        ht_r = hpool.tile([P, DT, THS], dt.float32r, tag="ht_r")
        encT_r = bpool.tile([P, ET, S], dt.float32r, tag="encT")
        enc_sb = bpool.tile([P, ST, E], dt.bfloat16, tag="enc")
        wo = wpool.tile([P, CT, D], dt.bfloat16)
        hb_sb = wpool.tile([P, BPC, TT], dt.float32)

        a_ap = A_r.ap().rearrange("(dt p) e -> p dt e", p=P)
        ht_ap0 = HT_r.ap()[0].rearrange("(dt p) t -> p dt t", p=P)
        wo_ap = WoT.ap().rearrange("(ct p) d -> p ct d", p=P)
        with tc.high_priority():
            nc.scalar.dma_start(hb_sb[:, 0, :],
                                hb.ap()[0].rearrange("(tt p) -> p tt", p=P))
            # mm1 inputs: H^T and A dt-slices alternate across both queues.
            for dti in range(DT):
                qh = nc.sync if dti % 2 == 0 else nc.scalar
                qa = nc.scalar if dti % 2 == 0 else nc.sync
                qh.dma_start(ht_r[:, dti, :], ht_ap0[:, dti, ts(0, THS)])
                qa.dma_start(a_r[:, dti, :], a_ap[:, dti, :])
            # mm2 inputs: Enc^T s-halves, one per queue.
            encT_ap0 = EncT_r.ap()[0].rearrange("(et p) s -> p et s", p=P)
            nc.sync.dma_start(encT_r[:, :, ts(0, 512)], encT_ap0[:, :, ts(0, 512)])
            nc.scalar.dma_start(encT_r[:, :, ts(1, 512)], encT_ap0[:, :, ts(1, 512)])
            # mm3 input for b0.
            nc.sync.dma_start(enc_sb[:],
                              Enc.ap()[0].rearrange("(st p) e -> p st e", p=P))

        pending = None  # deferred mm4 work: (b, th, ct_sb, ht_bf)

        def emit_mm4(job, post_tl=None):
            pb, pth, p_ct, p_ht = job
            for tl in range(NTL):
                tt = pth * NTL + tl
                h_sb = work.tile([P, D], dt.bfloat16, tag="h_sb")
                for dc in range(2):
                    hacc = psA.tile([P, 512], dt.float32, tag="psA")
                    for ci in range(ET):
                        nc.tensor.matmul(hacc[:], p_ct[:, ci, ts(tl, P)],
                                         wo[:, ci, ts(dc, 512)],
                                         start=(ci == 0), stop=False)
                    for ci in range(DT):
                        nc.tensor.matmul(hacc[:], p_ht[:, ci, ts(tl, P)],
                                         wo[:, ET + ci, ts(dc, 512)],
                                         start=False, stop=(ci == DT - 1))
                    nc.scalar.activation(h_sb[:, ts(dc, 512)], hacc[:], AF.Tanh)
                nc.scalar.dma_start(out_h.ap()[pb, ts(tt, P), :], h_sb[:])
                if post_tl is not None and tl in post_tl:
                    post_tl[tl]()

        for b in range(BPC):
            for th in range(TH):
                if not (b == 0 and th == 0):
                    # Next H^T ahead of everything else on the sync queue so
                    # mm1 streams immediately at iteration start.
                    ht_r = hpool.tile([P, DT, THS], dt.float32r, tag="ht_r")
                    ht_ap = HT_r.ap()[b].rearrange("(dt p) t -> p dt t", p=P)
                    for dti in range(DT):
                        nc.sync.dma_start(ht_r[:, dti, :],
                                          ht_ap[:, dti, ts(th, THS)])
                if b > 0 and th == 0:
                    # Next batch's Enc/Enc^T: mostly on the sync queue (the
                    # scalar queue carries this iteration's output writes and
                    # W^T transposes, which must not sit behind 6MB of input).
                    enc_sb = bpool.tile([P, ST, E], dt.bfloat16, tag="enc")
                    enc_ap = Enc.ap()[b].rearrange("(st p) e -> p st e", p=P)
                    encT_r = bpool.tile([P, ET, S], dt.float32r, tag="encT")
                    encT_ap = EncT_r.ap()[b].rearrange("(et p) s -> p et s", p=P)
                    nc.sync.dma_start(encT_r[:, :, ts(0, 512)],
                                      encT_ap[:, :, ts(0, 512)])
                    nc.scalar.dma_start(encT_r[:, :, ts(1, 512)],
                                        encT_ap[:, :, ts(1, 512)])
                    nc.sync.dma_start(enc_sb[:], enc_ap[:])
                    nc.scalar.dma_start(
                        hb_sb[:, b, :], hb.ap()[b].rearrange("(tt p) -> p tt", p=P))

                # ---- mm1: M1T[e, t] = sum_d A[d,e] * HT[d,t] (fp32r)
                # dt-outer over et-pairs: streams with the per-dt input DMAs.
                m1_r = hpool.tile([P, ET, THS], dt.float32r, tag="m1_r")
                for ep in range(ET // 2):
                    acc0 = psA.tile([P, 512], dt.float32, tag="psA")
                    acc1 = psA.tile([P, 512], dt.float32, tag="psA")
                    for dti in range(DT):
                        st_, sp_ = (dti == 0), (dti == DT - 1)
                        nc.tensor.matmul(acc0[:], a_r[:, dti, ts(2 * ep, P)],
                                         ht_r[:, dti, :], start=st_, stop=sp_)
                        nc.tensor.matmul(acc1[:], a_r[:, dti, ts(2 * ep + 1, P)],
                                         ht_r[:, dti, :], start=st_, stop=sp_)
                    nc.vector.tensor_copy(m1_r[:, 2 * ep, :], acc0[:])
                    nc.vector.tensor_copy(m1_r[:, 2 * ep + 1, :], acc1[:])

                # bf16 copy of H^T for mm4 (deferred one iteration, so this
                # load is never startup-critical).
                ht_bf = h2pool.tile([P, DT, THS], dt.bfloat16, tag="ht_bf")
                nc.sync.dma_start(
                    ht_bf[:],
                    HT_bf.ap()[b].rearrange("(dt p) t -> p dt t", p=P)[:, :, ts(th, THS)])

                # ---- mm2 + softmax per t-tile; W^T transposes on the PE,
                # emitted one tl late so each reads a finished softmax tile.
                wt_sb = hpool.tile([P, ST, THS], dt.bfloat16, tag="wt")
                wbfs = [None] * NTL

                def emit_transpose(tl):
                    for st in range(ST):
                        trp = psT.tile([P, P], dt.bfloat16, tag="psT")
                        nc.tensor.transpose(trp[:], wbfs[tl][:, ts(st, P)],
                                            ident[:])
                        nc.vector.tensor_copy(wt_sb[:, st, ts(tl, P)], trp[:])

                for tl in range(NTL):
                    tt = th * NTL + tl
                    G = psG.tile([P, S], dt.float32, tag="psG")
                    for sc in range(2):
                        for et in range(ET):
                            nc.tensor.matmul(
                                G[:, ts(sc, 512)],
                                m1_r[:, et, ts(tl, P)],
                                encT_r[:, et, ts(sc, 512)],
                                start=(et == 0), stop=(et == ET - 1))
                    # softmax is shift-invariant: work on ener = G + hb in
                    # SBUF fp32; the G psum banks free after this one copy.
                    ener32 = work.tile([P, S], dt.float32, tag="ener32")
                    nc.scalar.activation(ener32[:], G[:], AF.Identity,
                                         bias=hb_sb[:, b, tt:tt + 1], scale=1.0)
                    ener_bf = work4.tile([P, S], dt.bfloat16, tag="ener_bf")
                    nc.scalar.copy(ener_bf[:], ener32[:])
                    nc.scalar.dma_start(out_e.ap()[b, ts(tt, P), :], ener_bf[:])
                    negmax = work.tile([P, 1], dt.float32, tag="negmax")
                    nc.vector.reduce_max(negmax[:], ener32[:],
                                         axis=mybir.AxisListType.X, negate=True)
                    pexp = work.tile([P, S], dt.bfloat16, tag="pexp")
                    sume = work.tile([P, 1], dt.float32, tag="sume")
                    nc.scalar.activation(pexp[:], ener32[:], AF.Exp,
                                         bias=negmax[:], scale=1.0,
                                         accum_out=sume[:])
                    rec = work.tile([P, 1], dt.float32, tag="rec")
                    nc.vector.reciprocal(rec[:], sume[:])
                    wbf = work3.tile([P, S], dt.bfloat16, tag="wbf")
                    nc.vector.tensor_scalar_mul(wbf[:], in0=pexp[:], scalar1=rec[:])
                    nc.scalar.dma_start(out_w.ap()[b, ts(tt, P), :], wbf[:])
                    wbfs[tl] = wbf
                    if tl >= 2:
                        emit_transpose(tl - 2)

                if b == 0 and th == 0:
                    # W_out is first needed by the deferred mm4 an iteration
                    # from now; load it behind this iteration's outputs.
                    for dc in range(2):
                        nc.scalar.dma_start(wo[:, :, ts(dc, 512)],
                                            wo_ap[:, :, ts(dc, 512)])

                # ---- deferred mm4 of the previous iteration covers the
                # softmax/transpose tail of this one; the last two
                # transposes slot in between mm4's first t-tiles, giving the
                # softmax chain maximal slack before the PE needs W^T.
                if pending is not None:
                    emit_mm4(pending, post_tl={
                        0: lambda: emit_transpose(NTL - 2),
                        1: lambda: emit_transpose(NTL - 1),
                    })
                else:
                    warm(24)  # iteration 0: nothing to cover the tail with
                    emit_transpose(NTL - 2)
                    warm(8)
                    emit_transpose(NTL - 1)

                # ---- mm3: CT[e', t] = sum_s Enc[s,e'] * WT[s,t]
                ct_sb = hpool.tile([P, ET, THS], dt.bfloat16, tag="ct")
                for e2 in range(ET):
                    cacc = psA.tile([P, 512], dt.float32, tag="psA")
                    for st in range(ST):
                        nc.tensor.matmul(cacc[:], enc_sb[:, st, ts(e2, P)],
                                         wt_sb[:, st, :],
                                         start=(st == 0), stop=(st == ST - 1))
                    nc.scalar.copy(ct_sb[:, e2, :], cacc[:])

                pending = (b, th, ct_sb, ht_bf)

        warm(16)  # cover the ct-copy latency so the final mm4 stays at 8/8
        emit_mm4(pending)

    nc.compile()
    return nc


def kernel(hidden, encoder_outputs, W_attn, b_attn, W_out):
    global _cached, LAST_EXEC_NS
    hidden = np.asarray(hidden, dtype=np.float32)
    encoder_outputs = np.asarray(encoder_outputs, dtype=np.float32)
    W_attn = np.asarray(W_attn, dtype=np.float32)
    b_attn = np.asarray(b_attn, dtype=np.float32)
    W_out = np.asarray(W_out, dtype=np.float32)

    if TRACE:
        _install_trace_shim()
    if _cached is None:
        _cached = _build()
    nc = _cached
    from concourse.bass_utils import run_bass_kernel_spmd

    WoT = np.ascontiguousarray(W_out.T).astype(BF16)
    hb_full = (hidden.reshape(B * T, D) @ b_attn).reshape(B, T).astype(np.float32)

    in_maps = []
    for c in range(NCORES):
        sl = slice(BPC * c, BPC * (c + 1))
        h = hidden[sl]
        enc = encoder_outputs[sl]
        HT = np.ascontiguousarray(h.transpose(0, 2, 1))
        EncT = np.ascontiguousarray(enc.transpose(0, 2, 1))
        in_maps.append({
            "A_r": W_attn, "WoT": WoT,
            "HT_r": HT, "HT_bf": HT.astype(BF16),
            "EncT_r": EncT,
            "Enc": enc.astype(BF16),
            "hb": np.ascontiguousarray(hb_full[sl]),
        })

    res = run_bass_kernel_spmd(nc, in_maps, core_ids=list(range(NCORES)),
                               trace=TRACE)
    LAST_EXEC_NS = res.exec_time_ns

    h_tilde = np.concatenate(
        [np.asarray(r["out_h"], dtype=np.float32) for r in res.results], axis=0)
    attn_weights = np.concatenate(
        [np.asarray(r["out_w"], dtype=np.float32) for r in res.results], axis=0)
    attn_energies = np.concatenate(
        [np.asarray(r["out_e"], dtype=np.float32) for r in res.results], axis=0)
    return h_tilde, attn_weights, attn_energies


# revision 37
# speedup vs baseline: 1.2247x; 1.2247x over previous
"""Luong-style attention (B=16, T=S=E=D=1024) on 8 TRN2 NeuronCores.

Data-parallel over batch: 2 batches per core, no collectives. Per batch:

    M1   = H @ A            (T,E)     [A = W_attn]
    G    = M1 @ Enc^T       (T,S)     energies minus the row-constant H@b term
    ener = G + (H@b)[:,None]          (output attn_energies)
    W    = softmax_rows(G)            (== softmax(ener); bias is row-constant)
    C^T  = Enc(stationary) @ W^T      weighted context, transposed via DMA XBAR
    h    = tanh([C|H] @ W_out^T)      via lhsT = [C^T; H^T]

mm1/mm2 run in float32r (softmax input needs ~2^-13 operand precision),
mm3/mm4 in bf16 (fp8 was measured at 2-4.5e-2 final error — too coarse).

Measured on the 8-core chip: ~347-355us HW exec (baseline 387us),
rel err <= 3.3e-3. Tensor engine ~91% busy; steady-state matmul
start-to-start is 215-235ns for 512-col tiles (2.25-2.4 GHz; the upper
end of the range is the uncontrollable P0 power downclock).

Structure (vs the 387us baseline):
  - mm4 is software-pipelined one (b,th) iteration late, so its 27us of PE
    work covers the softmax/W^T-transpose latency of the current iteration
    (otherwise mm3 stalls on softmax(tl3) and HAM downclocks the PE).
  - W^T via PE identity transposes, emitted one tl late so each waits on an
    already-computed softmax tile. (The DMA XBAR transpose was tried on both
    HWDGE queues: on sync it races with input loads on HW and corrupts
    wt_sb; on scalar its serialization guard stalls the ACT stream 7-20us
    per transpose. Do not revisit.)
  - Softmax works on an SBUF fp32 copy of the energies (softmax is
    shift-invariant, so ener = G + hb serves directly); the PSUM G banks
    free after one ACT copy -> psG bufs=2 suffices.
  - mm1 is dt-outer over et-pairs with per-dt-slice DMAs of A^T/H^T
    interleaved across both HWDGE queues; the PE starts after ~1.5MB lands.
  - Outputs are bf16 on the wire (host upcasts).
"""

import os
import numpy as np
import ml_dtypes

B, T, S, E, D = 16, 1024, 1024, 1024, 1024
P = 128
NCORES = 8
BPC = B // NCORES
TH = 2
THS = T // TH
ET = E // P
DT = D // P
ST = S // P
TT = T // P
CT = (E + D) // P
NTL = THS // P

BF16 = ml_dtypes.bfloat16

TRACE = bool(os.environ.get("BASS_KERNEL_TRACE"))
LAST_EXEC_NS = None
_cached = None


def _install_trace_shim():
    import sys, types
    import antenv
    if getattr(antenv, "axon_hooks", None) is not None:
        return
    mod = types.ModuleType("antenv.axon_hooks")
    state = {"hook": None}
    mod.set_axon_ntff_profile_hook = lambda h: state.__setitem__("hook", h)
    mod.get_axon_ntff_profile_hook = lambda: state["hook"]
    sys.modules["antenv.axon_hooks"] = mod
    antenv.axon_hooks = mod
    try:
        from trn_agent_boot.trn_boot import _ntff_profile_via_ctypes
        mod.set_axon_ntff_profile_hook(
            _ntff_profile_via_ctypes("/opt/axon/libaxon_pjrt.so"))
    except Exception:
        pass
    import concourse.bass_utils as bu
    bu.upload_artifacts = lambda tmpdir: "local://" + tmpdir


def _build():
    import concourse.bass as bass
    import concourse.bacc as bacc
    import concourse.mybir as mybir
    import concourse.tile as tile
    from contextlib import ExitStack

    dt = mybir.dt
    ts = bass.ts
    AF = mybir.ActivationFunctionType

    nc = bacc.Bacc("TRN2", target_bir_lowering=False, debug=False)

    A_r = nc.declare_dram_parameter("A_r", [D, E], dt.float32r, isOutput=False)
    WoT = nc.declare_dram_parameter("WoT", [E + D, D], dt.bfloat16, isOutput=False)
    HT_r = nc.declare_dram_parameter("HT_r", [BPC, D, T], dt.float32r, isOutput=False)
    HT_bf = nc.declare_dram_parameter("HT_bf", [BPC, D, T], dt.bfloat16, isOutput=False)
    EncT_r = nc.declare_dram_parameter("EncT_r", [BPC, E, S], dt.float32r, isOutput=False)
    Enc = nc.declare_dram_parameter("Enc", [BPC, S, E], dt.bfloat16, isOutput=False)
    hb = nc.declare_dram_parameter("hb", [BPC, T], dt.float32, isOutput=False)
    out_h = nc.declare_dram_parameter("out_h", [BPC, T, D], dt.bfloat16, isOutput=True)
    out_w = nc.declare_dram_parameter("out_w", [BPC, T, S], dt.bfloat16, isOutput=True)
    out_e = nc.declare_dram_parameter("out_e", [BPC, T, S], dt.bfloat16, isOutput=True)

    with tile.TileContext(nc) as tc, ExitStack() as ctx:
        from concourse.masks import make_identity

        const = ctx.enter_context(tc.tile_pool(name="const", bufs=1))
        wpool = ctx.enter_context(tc.tile_pool(name="wpool", bufs=1))
        bpool = ctx.enter_context(tc.tile_pool(name="bpool", bufs=1))
        hpool = ctx.enter_context(tc.tile_pool(name="hpool", bufs=1))
        h2pool = ctx.enter_context(tc.tile_pool(name="h2pool", bufs=2))
        work = ctx.enter_context(tc.tile_pool(name="work", bufs=2))
        work3 = ctx.enter_context(tc.tile_pool(name="work3", bufs=3))
        work4 = ctx.enter_context(tc.tile_pool(name="work4", bufs=4))
        psA = ctx.enter_context(tc.tile_pool(name="psA", bufs=2, space="PSUM"))
        psG = ctx.enter_context(tc.tile_pool(name="psG", bufs=2, space="PSUM"))
        psT = ctx.enter_context(tc.tile_pool(name="psT", bufs=2, space="PSUM"))

        ident = const.tile([P, P], dt.bfloat16)
        make_identity(nc, ident[:])
        warm_a = const.tile([P, P], dt.bfloat16)
        warm_r = const.tile([P, 512], dt.bfloat16)
        nc.gpsimd.memset(warm_a[:], 0.0)
        nc.gpsimd.memset(warm_r[:], 0.0)

        def warm(n):
            # HAM/idle filler: keeps the PE streaming while DMAs land.
            wp = psA.tile([P, 512], dt.float32, tag="psA")
            for wi in range(n):
                nc.tensor.matmul(wp[:], warm_a[:], warm_r[:],
                                 start=(wi == 0), stop=(wi == n - 1))

        warm(24)

        # --- startup-critical DMAs, interleaved across the two HWDGE queues
        a_r = wpool.tile([P, DT, E], dt.float32r)
        ht_r = hpool.tile([P, DT, THS], dt.float32r, tag="ht_r")
        encT_r = bpool.tile([P, ET, S], dt.float32r, tag="encT")
        enc_sb = bpool.tile([P, ST, E], dt.bfloat16, tag="enc")
        wo = wpool.tile([P, CT, D], dt.bfloat16)
        hb_sb = wpool.tile([P, BPC, TT], dt.float32)

        a_ap = A_r.ap().rearrange("(dt p) e -> p dt e", p=P)
        ht_ap0 = HT_r.ap()[0].rearrange("(dt p) t -> p dt t", p=P)
        wo_ap = WoT.ap().rearrange("(ct p) d -> p ct d", p=P)
        with tc.high_priority():
            nc.scalar.dma_start(hb_sb[:, 0, :],
                                hb.ap()[0].rearrange("(tt p) -> p tt", p=P))
            # mm1 inputs: H^T and A dt-slices alternate across both queues.
            for dti in range(DT):
                qh = nc.sync if dti % 2 == 0 else nc.scalar
                qa = nc.scalar if dti % 2 == 0 else nc.sync
                qh.dma_start(ht_r[:, dti, :], ht_ap0[:, dti, ts(0, THS)])
                qa.dma_start(a_r[:, dti, :], a_ap[:, dti, :])
            # mm2 inputs: Enc^T s-halves, one per queue.
            encT_ap0 = EncT_r.ap()[0].rearrange("(et p) s -> p et s", p=P)
            nc.sync.dma_start(encT_r[:, :, ts(0, 512)], encT_ap0[:, :, ts(0, 512)])
            nc.scalar.dma_start(encT_r[:, :, ts(1, 512)], encT_ap0[:, :, ts(1, 512)])
            # mm3 input for b0.
            nc.sync.dma_start(enc_sb[:],
                              Enc.ap()[0].rearrange("(st p) e -> p st e", p=P))

        pending = None  # deferred mm4 work: (b, th, ct_sb, ht_bf)

        def emit_mm4(job, post_tl=None, final=False):
            pb, pth, p_ct, p_ht = job
            for tl in range(NTL):
                tt = pth * NTL + tl
                h_sb = work.tile([P, D], dt.bfloat16, tag="h_sb")
                for dc in range(2):
                    hacc = psA.tile([P, 512], dt.float32, tag="psA")
                    for ci in range(ET):
                        nc.tensor.matmul(hacc[:], p_ct[:, ci, ts(tl, P)],
                                         wo[:, ci, ts(dc, 512)],
                                         start=(ci == 0), stop=False)
                    for ci in range(DT):
                        nc.tensor.matmul(hacc[:], p_ht[:, ci, ts(tl, P)],
                                         wo[:, ET + ci, ts(dc, 512)],
                                         start=False, stop=(ci == DT - 1))
                    nc.scalar.activation(h_sb[:, ts(dc, 512)], hacc[:], AF.Tanh)
                hq = nc.sync if (final and tl % 2 == 0) else nc.scalar
                hq.dma_start(out_h.ap()[pb, ts(tt, P), :], h_sb[:])
                if post_tl is not None and tl in post_tl:
                    post_tl[tl]()

        for b in range(BPC):
            for th in range(TH):
                if not (b == 0 and th == 0):
                    # Next H^T ahead of everything else on the sync queue so
                    # mm1 streams immediately at iteration start.
                    ht_r = hpool.tile([P, DT, THS], dt.float32r, tag="ht_r")
                    ht_ap = HT_r.ap()[b].rearrange("(dt p) t -> p dt t", p=P)
                    for dti in range(DT):
                        nc.sync.dma_start(ht_r[:, dti, :],
                                          ht_ap[:, dti, ts(th, THS)])
                if b > 0 and th == 0:
                    # Next batch's Enc/Enc^T: mostly on the sync queue (the
                    # scalar queue carries this iteration's output writes and
                    # W^T transposes, which must not sit behind 6MB of input).
                    enc_sb = bpool.tile([P, ST, E], dt.bfloat16, tag="enc")
                    enc_ap = Enc.ap()[b].rearrange("(st p) e -> p st e", p=P)
                    encT_r = bpool.tile([P, ET, S], dt.float32r, tag="encT")
                    encT_ap = EncT_r.ap()[b].rearrange("(et p) s -> p et s", p=P)
                    nc.sync.dma_start(encT_r[:, :, ts(0, 512)],
                                      encT_ap[:, :, ts(0, 512)])
                    nc.scalar.dma_start(encT_r[:, :, ts(1, 512)],
                                        encT_ap[:, :, ts(1, 512)])
                    nc.sync.dma_start(enc_sb[:], enc_ap[:])
                    nc.scalar.dma_start(
                        hb_sb[:, b, :], hb.ap()[b].rearrange("(tt p) -> p tt", p=P))

                # ---- mm1: M1T[e, t] = sum_d A[d,e] * HT[d,t] (fp32r)
                # dt-outer over et-pairs: streams with the per-dt input DMAs.
                # Accumulator pairs live in psG tiles (idle during mm1): 4
                # banks of rotation, so pass p+1 accumulates while pass p's
                # PSUM->SBUF copies drain (psA's 2 banks stalled each pass
                # boundary ~1us during the DMA-paced first iteration).
                # (Iteration 0 is DMA-paced and runs partly at HAM 4/8;
                # pacing filler matmuls between steps was tried and cost more
                # PE time than the downclock it prevented.)
                m1_r = hpool.tile([P, ET, THS], dt.float32r, tag="m1_r")
                for ep in range(ET // 2):
                    gacc = psG.tile([P, S], dt.float32, tag="psG")
                    acc0 = gacc[:, ts(0, 512)]
                    acc1 = gacc[:, ts(1, 512)]
                    for dti in range(DT):
                        st_, sp_ = (dti == 0), (dti == DT - 1)
                        nc.tensor.matmul(acc0, a_r[:, dti, ts(2 * ep, P)],
                                         ht_r[:, dti, :], start=st_, stop=sp_)
                        nc.tensor.matmul(acc1, a_r[:, dti, ts(2 * ep + 1, P)],
                                         ht_r[:, dti, :], start=st_, stop=sp_)
                    nc.vector.tensor_copy(m1_r[:, 2 * ep, :], acc0)
                    nc.vector.tensor_copy(m1_r[:, 2 * ep + 1, :], acc1)

                # bf16 copy of H^T for mm4 (deferred one iteration, so this
                # load is never startup-critical).
                ht_bf = h2pool.tile([P, DT, THS], dt.bfloat16, tag="ht_bf")
                nc.sync.dma_start(
                    ht_bf[:],
                    HT_bf.ap()[b].rearrange("(dt p) t -> p dt t", p=P)[:, :, ts(th, THS)])

                # ---- mm2 + softmax per t-tile; W^T transposes on the PE,
                # emitted one tl late so each reads a finished softmax tile.
                wt_sb = hpool.tile([P, ST, THS], dt.bfloat16, tag="wt")
                wbfs = [None] * NTL

                def emit_transpose(tl):
                    for st in range(ST):
                        trp = psT.tile([P, P], dt.bfloat16, tag="psT")
                        nc.tensor.transpose(trp[:], wbfs[tl][:, ts(st, P)],
                                            ident[:])
                        nc.vector.tensor_copy(wt_sb[:, st, ts(tl, P)], trp[:])

                for tl in range(NTL):
                    tt = th * NTL + tl
                    G = psG.tile([P, S], dt.float32, tag="psG")
                    for sc in range(2):
                        for et in range(ET):
                            nc.tensor.matmul(
                                G[:, ts(sc, 512)],
                                m1_r[:, et, ts(tl, P)],
                                encT_r[:, et, ts(sc, 512)],
                                start=(et == 0), stop=(et == ET - 1))
                    # softmax is shift-invariant: work on ener = G + hb in
                    # SBUF fp32; the G psum banks free after this one copy.
                    ener32 = work.tile([P, S], dt.float32, tag="ener32")
                    nc.scalar.activation(ener32[:], G[:], AF.Identity,
                                         bias=hb_sb[:, b, tt:tt + 1], scale=1.0)
                    # Last iteration's outputs drain on the idle sync queue so
                    # the end-of-kernel barrier isn't stuck behind a scalar
                    # queue backlog.
                    oq = nc.sync if (b == BPC - 1 and th == TH - 1) else nc.scalar
                    ener_bf = work4.tile([P, S], dt.bfloat16, tag="ener_bf")
                    nc.scalar.copy(ener_bf[:], ener32[:])
                    oq.dma_start(out_e.ap()[b, ts(tt, P), :], ener_bf[:])
                    negmax = work.tile([P, 1], dt.float32, tag="negmax")
                    nc.vector.reduce_max(negmax[:], ener32[:],
                                         axis=mybir.AxisListType.X, negate=True)
                    pexp = work.tile([P, S], dt.bfloat16, tag="pexp")
                    sume = work.tile([P, 1], dt.float32, tag="sume")
                    nc.scalar.activation(pexp[:], ener32[:], AF.Exp,
                                         bias=negmax[:], scale=1.0,
                                         accum_out=sume[:])
                    rec = work.tile([P, 1], dt.float32, tag="rec")
                    nc.vector.reciprocal(rec[:], sume[:])
                    wbf = work3.tile([P, S], dt.bfloat16, tag="wbf")
                    nc.vector.tensor_scalar_mul(wbf[:], in0=pexp[:], scalar1=rec[:])
                    oq.dma_start(out_w.ap()[b, ts(tt, P), :], wbf[:])
                    wbfs[tl] = wbf
                    if tl >= 2:
                        emit_transpose(tl - 2)

                if b == 0 and th == 0:
                    # W_out is first needed by the deferred mm4 an iteration
                    # from now; load it behind this iteration's outputs.
                    for dc in range(2):
                        nc.scalar.dma_start(wo[:, :, ts(dc, 512)],
                                            wo_ap[:, :, ts(dc, 512)])

                # ---- deferred mm4 of the previous iteration covers the
                # softmax/transpose tail of this one; the last two
                # transposes slot in between mm4's first t-tiles, giving the
                # softmax chain maximal slack before the PE needs W^T.
                if pending is not None:
                    emit_mm4(pending, post_tl={
                        0: lambda: emit_transpose(NTL - 2),
                        1: lambda: emit_transpose(NTL - 1),
                    })
                else:
                    warm(24)  # iteration 0: nothing to cover the tail with
                    emit_transpose(NTL - 2)
                    warm(8)
                    emit_transpose(NTL - 1)

                # ---- mm3: CT[e', t] = sum_s Enc[s,e'] * WT[s,t]
                ct_sb = hpool.tile([P, ET, THS], dt.bfloat16, tag="ct")
                for e2 in range(ET):
                    cacc = psA.tile([P, 512], dt.float32, tag="psA")
                    for st in range(ST):
                        nc.tensor.matmul(cacc[:], enc_sb[:, st, ts(e2, P)],
                                         wt_sb[:, st, :],
                                         start=(st == 0), stop=(st == ST - 1))
                    nc.scalar.copy(ct_sb[:, e2, :], cacc[:])

                pending = (b, th, ct_sb, ht_bf)

        warm(16)  # cover the ct-copy latency so the final mm4 stays at 8/8
        emit_mm4(pending, final=True)

    nc.compile()
    return nc


def kernel(hidden, encoder_outputs, W_attn, b_attn, W_out):
    global _cached, LAST_EXEC_NS
    hidden = np.asarray(hidden, dtype=np.float32)
    encoder_outputs = np.asarray(encoder_outputs, dtype=np.float32)
    W_attn = np.asarray(W_attn, dtype=np.float32)
    b_attn = np.asarray(b_attn, dtype=np.float32)
    W_out = np.asarray(W_out, dtype=np.float32)

    if TRACE:
        _install_trace_shim()
    if _cached is None:
        _cached = _build()
    nc = _cached
    from concourse.bass_utils import run_bass_kernel_spmd

    WoT = np.ascontiguousarray(W_out.T).astype(BF16)
    hb_full = (hidden.reshape(B * T, D) @ b_attn).reshape(B, T).astype(np.float32)

    in_maps = []
    for c in range(NCORES):
        sl = slice(BPC * c, BPC * (c + 1))
        h = hidden[sl]
        enc = encoder_outputs[sl]
        HT = np.ascontiguousarray(h.transpose(0, 2, 1))
        EncT = np.ascontiguousarray(enc.transpose(0, 2, 1))
        in_maps.append({
            "A_r": W_attn, "WoT": WoT,
            "HT_r": HT, "HT_bf": HT.astype(BF16),
            "EncT_r": EncT,
            "Enc": enc.astype(BF16),
            "hb": np.ascontiguousarray(hb_full[sl]),
        })

    res = run_bass_kernel_spmd(nc, in_maps, core_ids=list(range(NCORES)),
                               trace=TRACE)
    LAST_EXEC_NS = res.exec_time_ns

    h_tilde = np.concatenate(
        [np.asarray(r["out_h"], dtype=np.float32) for r in res.results], axis=0)
    attn_weights = np.concatenate(
        [np.asarray(r["out_w"], dtype=np.float32) for r in res.results], axis=0)
    attn_energies = np.concatenate(
        [np.asarray(r["out_e"], dtype=np.float32) for r in res.results], axis=0)
    return h_tilde, attn_weights, attn_energies
